# revision 33
# baseline (speedup 1.0000x reference)
"""Bass/Trainium2 kernel for blockwise cross-attention.

Math (per batch element b, per 16-row block):
  out1 = softmax(q1 k2^T / sqrt(E)) @ v2,  out2 = softmax(q2 k1^T / sqrt(E)) @ v1
with q = x Wq^T + bq etc.  Since softmax is shift-invariant along the key
axis, the q-side bias terms drop and
  softmax(q1 k2^T / s) == softmax(x1 A x2^T + 1 (x2 c)^T)
with A = Wq^T Wk / s and c = Wk^T bq / s precomputed on the host.  This
replaces 6 big projections with 4 (z = x A^T fused for both q&k roles, plus
v' = x Wv^T).  The v bias folds in exactly because softmax rows sum to 1.

The z projection runs in fp8 (e4m3) DoubleRow mode: each DR matmul contracts
two 128-chunks at once, halving the z instruction count vs bf16.  The fp8
noise (A and x quantization) is softmax-damped and fits the error budget.
Everything else (x, Wv, z storage, v, attn, mfac, outputs) is fp16: same PE
rate as bf16 but 3 more mantissa bits, which keeps total error BELOW the old
bf16 baseline while outputs shrink to 2 bytes (half the write traffic).
fp8 for scores or v was measured to breach the 2e-2 budget.

Emission per group: z(s0) v(s0) z(s1) v(s1) | scores | outs(g-1).  Keeping
the PE stream free of stalls matters doubly because any stall drops the PE to
its low p-state (~1.2GHz) for the next ~3us.  PSUM: psA=3 (z+v), psS=2
(scores), psO=3 (out) — psO=3 decouples the out matmuls from the o-scale
copies that queue behind the softmax chain on ACT/DVE.  For the final group
the scores are emitted before v(s1) so the softmax chain overlaps the last v
projections, and its stores go per-window so the drain pipelines.

All input loads issue from the otherwise-idle SP engine (a dma_start's
semaphore wait blocks the issuing engine's instruction stream, so loads must
not issue from ACT/DVE); mid-stream output stores ride the idle gpsimd SWDGE
queue so they can never head-of-line block loads, and the final group's
stores use the by-then-idle SP queue whose semaphore ops are ~5x faster.

Sharding: pure data-parallel — batch B=8, one batch element per NeuronCore.
"""

import math
import sys

if "/opt/trn_rl_repo" not in sys.path:
    sys.path.insert(0, "/opt/trn_rl_repo")

import numpy as np
import ml_dtypes

F16 = np.float16
F8 = ml_dtypes.float8_e4m3
BLOCK = 16  # attention block size (ceil(S**(2/3)) blocks => 16 for S=4096)
SA = 2048.0  # scale for A^T into fp8 range (A elems ~4e-4)


def _build_nc(S: int, E: int):
    from contextlib import ExitStack

    import concourse.bass as bass
    import concourse.tile as tile
    from concourse import bacc, mybir

    f32 = mybir.dt.float32
    f16 = mybir.dt.float16
    fp8 = mybir.dt.float8e4
    P = 128
    GROUP = 512  # rows per group
    G = S // GROUP
    NCH = E // P  # e-chunks (4)
    NW = GROUP // P  # windows per group (4)
    assert S % GROUP == 0 and E == 512

    DR = mybir.MatmulPerfMode.DoubleRow
    Exp = mybir.ActivationFunctionType.Exp

    nc = bacc.Bacc("TRN2", debug=False)

    # host pre-packs x^T as [p, (s, c), t] so a group load is a 3-dim slice
    x_dram = nc.dram_tensor("xt", [P, 2 * NCH, S], f16, kind="ExternalInput").ap()
    x8_dram = nc.dram_tensor("x8", [P, 2 * NCH, S], fp8, kind="ExternalInput").ap()
    athi_dram = nc.dram_tensor("at_hi", [E, E], fp8, kind="ExternalInput").ap()
    wvt_dram = nc.dram_tensor("wvt", [E, E], f16, kind="ExternalInput").ap()
    # per-(state, group) post-exp factor M[q,k] = e^{t[k]} * [q,k same block]
    mf_dram = nc.dram_tensor("mfac", [2, G, P, GROUP], f16, kind="ExternalInput").ap()
    out_dram = [
        nc.dram_tensor("out1", [S, E], f16, kind="ExternalOutput").ap(),
        nc.dram_tensor("out2", [S, E], f16, kind="ExternalOutput").ap(),
    ]

    with ExitStack() as ctx:
        tc = ctx.enter_context(tile.TileContext(nc))

        consts = ctx.enter_context(tc.tile_pool(name="consts", bufs=1))
        xt_pool = ctx.enter_context(tc.tile_pool(name="xt", bufs=2))
        x8_pool = ctx.enter_context(tc.tile_pool(name="x8", bufs=2))
        z_pool = ctx.enter_context(tc.tile_pool(name="z", bufs=2))
        v_pool = ctx.enter_context(tc.tile_pool(name="v", bufs=2))
        mf_pool = ctx.enter_context(tc.tile_pool(name="mf", bufs=2))
        sm_pool = ctx.enter_context(tc.tile_pool(name="sm", bufs=3))
        an_pool = ctx.enter_context(tc.tile_pool(name="an", bufs=8))
        o_pool = ctx.enter_context(tc.tile_pool(name="o", bufs=3))
        psA = ctx.enter_context(tc.tile_pool(name="psA", bufs=3, space="PSUM"))
        psS = ctx.enter_context(tc.tile_pool(name="psS", bufs=2, space="PSUM"))
        psO = ctx.enter_context(tc.tile_pool(name="psO", bufs=3, space="PSUM"))

        # --- constants (at_hi first: the first z matmul needs it; split in
        # chunk-pair halves so the first DR matmul starts after half the load)
        at_hi = consts.tile([P, NCH * E], fp8, name="athi", tag="athi")
        for cp in (0, 2):
            nc.sync.dma_start(
                at_hi.rearrange("p (c e) -> p c e", c=NCH)[:, cp : cp + 2, :],
                athi_dram.rearrange("(c p) e -> p c e", p=P)[:, cp : cp + 2, :],
            )
        wv_t = consts.tile([P, NCH * E], f16, name="wvt", tag="wvt")
        nc.scalar.dma_start(
            wv_t.rearrange("p (c e) -> p c e", c=NCH),
            wvt_dram.rearrange("(c p) e -> p c e", p=P),
        )

        def at_pair(cp, m):  # [128, 2, 128] c-pair view, e_out m-slice
            return at_hi.rearrange("p (c e) -> p c e", c=NCH)[
                :, cp : cp + 2, m * P : (m + 1) * P
            ]

        def wv_c(c):
            return wv_t[:, c * E : (c + 1) * E]

        st = {}  # per-group: (xt, zt, vt, mf)
        mv_n = [0]
        MV_PAT = [0, 1, 0, 1, 0, 1, 0, 1, 0, 1, 0, 0, 1, 0, 1, 0, 1, 0, 1, 0, 1, 0, 0, 0]

        def mv_copy(out_ap, in_ap):
            e = MV_PAT[mv_n[0] % len(MV_PAT)]
            mv_n[0] += 1
            if e == 0:
                nc.scalar.copy(out_ap, in_ap)
            else:
                nc.vector.tensor_copy(out_ap, in_ap)

        def mv_scale(out_ap, in_ap, scale_ap):
            e = MV_PAT[mv_n[0] % len(MV_PAT)]
            mv_n[0] += 1
            if e == 0:
                nc.scalar.mul(out_ap, in_ap, scale_ap)
            else:
                nc.vector.tensor_scalar_mul(out_ap, in_ap, scale_ap)

        sm = {}  # per-group: list of (attnT, rcp) per window

        def emit_loads(g):
            r0 = g * GROUP
            # per-state loads keep dependencies fine-grained: z(s) starts as
            # soon as its own x8 half lands (dma_start issue cost is ~flat).
            x8 = {}
            xt = {}
            mf = {}
            order = ("x8", "x8", "xt", "xt") if g == 0 else ("x8", "xt", "x8", "xt")
            seen = {"x8": 0, "xt": 0}
            for kind in order:
                s = seen[kind]; seen[kind] += 1
                if kind == "x8":
                    x8_tl = x8_pool.tile([P, NCH * GROUP], fp8, name=f"x8{s}", tag=f"x8{s}")
                    nc.sync.dma_start(
                        x8_tl.rearrange("p (c r) -> p c r", c=NCH),
                        x8_dram[:, s * NCH : (s + 1) * NCH, r0 : r0 + GROUP],
                    )
                    x8[s] = x8_tl
                else:
                    x_tl = xt_pool.tile([P, NCH * GROUP], f16, name=f"xt{s}", tag=f"xt{s}")
                    nc.sync.dma_start(
                        x_tl.rearrange("p (c r) -> p c r", c=NCH),
                        x_dram[:, s * NCH : (s + 1) * NCH, r0 : r0 + GROUP],
                    )
                    xt[s] = x_tl
            for s in range(2):
                mf_tl = mf_pool.tile([P, GROUP], f16, name=f"mf{s}", tag=f"mf{s}")
                nc.sync.dma_start(mf_tl[:], mf_dram[s, g])
                mf[s] = mf_tl
            st[g] = [xt, {}, {}, mf, x8]

        def emit_proj(g, scores_early=False):
            xt, zt, vt, mf, x8 = st[g]

            def x8_pair(s, cp):  # [128, 2, 512] fp8 c-pair view
                return x8[s].rearrange("p (c r) -> p c r", c=NCH)[:, cp : cp + 2, :]

            def xt_c(s, c):  # x^T chunk c: [128 e_in, 512 rows] f16
                return xt[s][:, c * GROUP : (c + 1) * GROUP]

            def do_z(s):
                # z_s^T m-chunk [128 e_out, GROUP rows] = SA * z, fp8 DR pairs
                for m in range(NCH):
                    z_ps = psA.tile([P, GROUP], f32, name="zps", tag="psA")
                    for n, cp in enumerate((0, 2)):
                        nc.tensor.matmul(
                            z_ps[:], at_pair(cp, m), x8_pair(s, cp),
                            start=(n == 0), stop=(n == 1), perf_mode=DR,
                        )
                    z_sb = z_pool.tile([P, GROUP], f16, name=f"zsb{s}{m}", tag=f"zsb{s}{m}")
                    mv_copy(z_sb[:], z_ps[:])
                    zt[s, m] = z_sb

            def do_v(s):
                # v'_s r-chunk [128 rows, E] = x @ Wv^T (bv added on host)
                for r in range(NW):
                    v_ps = psA.tile([P, E], f32, name="vps", tag="psA")
                    for c in range(NCH):
                        nc.tensor.matmul(
                            v_ps[:], xt_c(s, c)[:, r * P : (r + 1) * P], wv_c(c),
                            start=(c == 0), stop=(c == NCH - 1),
                        )
                    v_sb = v_pool.tile([P, E], f16, name=f"vsb{s}{r}", tag=f"vsb{s}{r}")
                    mv_copy(v_sb[:], v_ps[:])
                    vt[s, r] = v_sb

            if g == 0:
                do_z(0)
                do_z(1)
                do_v(0)
                do_v(1)
                return
            do_z(0)
            do_v(0)
            do_z(1)
            if scores_early:
                # final group: emit scores before v(s1) so the softmax chain
                # overlaps the last v projections and the drain is short
                emit_scores(g)
            do_v(1)

        def emit_scores(g):
            xt, zt, vt, mf, x8 = st[g]
            wins = []
            for w in range(NW):
                ws = slice(w * P, (w + 1) * P)
                # both directions' scores into one [128, 256] PSUM tile
                s_ps = psS.tile([P, 2 * P], f32, name="sps", tag="psS")
                for qs, ks in ((0, 1), (1, 0)):
                    dst = s_ps[:, qs * P : (qs + 1) * P]
                    for m in range(NCH):
                        nc.tensor.matmul(
                            dst,
                            xt[qs][:, m * GROUP + w * P : m * GROUP + (w + 1) * P],
                            zt[ks, m][:, ws],
                            start=(m == 0), stop=(m == NCH - 1),
                        )
                exp_sb = sm_pool.tile([P, 2 * P], f32, name="expsb", tag="expsb")
                nc.scalar.activation(exp_sb[:], s_ps[:], Exp, scale=1.0 / SA)
                # masked UNNORMALIZED attn = exp * M, fused row-sum (per dir)
                mskd = sm_pool.tile([P, 2 * P], f16, name="mskd", tag="mskd")
                rsum = an_pool.tile([P, 2], f32, name="rsum", tag="rsum")
                for qs, ks in ((0, 1), (1, 0)):
                    nc.vector.scalar_tensor_tensor(
                        mskd[:, qs * P : (qs + 1) * P],
                        exp_sb[:, qs * P : (qs + 1) * P], 1.0, mf[ks][:, ws],
                        op0=mybir.AluOpType.mult, op1=mybir.AluOpType.mult,
                        accum_out=rsum[:, qs : qs + 1],
                    )
                rcp = an_pool.tile([P, 2], f32, name="rcp", tag="rcp")
                nc.vector.reciprocal(rcp[:], rsum[:])
                attnT = an_pool.tile([P, 2 * P], f16, name="attnT", tag="attnT")
                nc.vector.transpose(attnT[:], mskd[:])
                wins.append((attnT, rcp))
            sm[g] = wins

        def emit_outs(g, last=False):
            r0 = g * GROUP
            xt, zt, vt, mf, x8 = st.pop(g)
            wins = sm.pop(g)
            o_sb = {
                qs: o_pool.tile([P, NW * E], f16, name=f"osb{qs}", tag=f"osb{qs}")
                for qs in range(2)
            }
            for w in range(NW):
                attnT, rcp = wins[w]
                for qs, ks in ((0, 1), (1, 0)):
                    o_ps = psO.tile([P, E], f32, name="ops", tag="psO")
                    nc.tensor.matmul(
                        o_ps[:], attnT[:, qs * P : (qs + 1) * P], vt[ks, w][:],
                        start=True, stop=True,
                    )
                    # out = (attn_unnorm @ v) * recip[q]; bv is added on host
                    mv_scale(o_sb[qs][:, w * E : (w + 1) * E], o_ps[:], rcp[:, qs : qs + 1])
            # one store per direction per group: [P, (w e)] -> rows w*P+p.
            # mid-stream stores ride the idle gpsimd SWDGE queue; the final
            # group stores per-window on the (by then idle) sync queue so the
            # drain pipelines with the remaining scale ops.
            if last:
                for qs in range(2):
                    for w in range(NW):
                        nc.sync.dma_start(
                            out_dram[qs][r0 + w * P : r0 + (w + 1) * P, :],
                            o_sb[qs][:, w * E : (w + 1) * E],
                        )
            else:
                for qs in range(2):
                    nc.gpsimd.dma_start(
                        out_dram[qs][r0 : r0 + GROUP, :].rearrange("(w p) e -> p w e", p=P),
                        o_sb[qs].rearrange("p (w e) -> p w e", w=NW),
                    )

        emit_loads(0)
        emit_proj(0)
        emit_scores(0)
        for g in range(1, G - 1):
            emit_loads(g)
            emit_proj(g)
            emit_scores(g)
            emit_outs(g - 1)
        emit_loads(G - 1)
        emit_proj(G - 1, scores_early=True)
        emit_outs(G - 2)
        emit_outs(G - 1, last=True)

    nc.compile()
    return nc


def _host_inputs(state1, state2, Wq, bq, Wk, bk, Wv, bv, S, E):
    """Build the per-core common (weight) arrays + per-core x arrays."""
    P = 128
    GROUP = 512
    G = S // GROUP
    scale = math.sqrt(E)
    Wq64 = np.asarray(Wq, np.float64)
    Wk64 = np.asarray(Wk, np.float64)
    # A = Wq^T Wk / scale ; device needs A^T = Wk^T Wq / scale  [e_in, e_out]
    atm = (Wk64.T @ Wq64 / scale).astype(np.float32)
    at_hi = (atm * SA).astype(F8)
    cvec = (Wk64.T @ np.asarray(bq, np.float64) / scale).astype(np.float32)  # [E]
    wvt = np.ascontiguousarray(np.asarray(Wv, np.float32).T).astype(F16)
    common = {
        "at_hi": np.ascontiguousarray(at_hi),
        "wvt": wvt,
    }
    # post-exp factor M[q, k] = [q, k in same 16-block] * e^{t[k]}
    idx = np.arange(P)
    kidx = np.arange(GROUP) % P
    pattern = (idx[:, None] // BLOCK == kidx[None, :] // BLOCK).astype(np.float32)
    x1 = np.asarray(state1, np.float32)
    x2 = np.asarray(state2, np.float32)
    B = x1.shape[0]
    per_core = []
    for b in range(B):
        mfac = np.empty((2, G, P, GROUP), np.float32)
        for s, x in ((0, x1[b]), (1, x2[b])):
            et = np.exp(x @ cvec).reshape(G, 1, GROUP)
            mfac[s] = pattern[None, :, :] * et
        # pack x^T [2, E, S] -> [P, (s, c), S] with e = c*P + p
        xts = np.stack([x1[b].T, x2[b].T])  # [2, E, S]
        NCH = E // P
        xp = np.ascontiguousarray(
            xts.reshape(2, NCH, P, S).transpose(2, 0, 1, 3)
        )  # [P, 2, NCH, S]
        xp = xp.reshape(P, 2 * NCH, S)
        per_core.append(
            {
                "xt": xp.astype(F16),
                "x8": xp.astype(F8),
                "mfac": mfac.astype(F16),
                **common,
            }
        )
    return per_core


_NC_CACHE = {}


def _get_nc(S, E):
    key = (S, E)
    if key not in _NC_CACHE:
        _NC_CACHE[key] = _build_nc(S, E)
    return _NC_CACHE[key]


def kernel(state1, state2, Wq, bq, Wk, bk, Wv, bv):
    from concourse.bass_utils import run_bass_kernel_spmd

    state1 = np.asarray(state1)
    B, S, E = state1.shape
    assert (B, S, E) == (8, 4096, 512), (B, S, E)

    nc = _get_nc(S, E)
    in_maps = _host_inputs(state1, state2, Wq, bq, Wk, bk, Wv, bv, S, E)
    res = run_bass_kernel_spmd(nc, in_maps, list(range(B)))
    bvf = np.asarray(bv, np.float32)[None, None, :]
    out1 = np.stack([res.results[b]["out1"].astype(np.float32) for b in range(B)]) + bvf
    out2 = np.stack([res.results[b]["out2"].astype(np.float32) for b in range(B)]) + bvf
    return out1, out2


if __name__ == "__main__":
    rng = np.random.default_rng(0)
    B, S, E = 8, 4096, 512
    ins = {
        "state1": rng.standard_normal((B, S, E), np.float32),
        "state2": rng.standard_normal((B, S, E), np.float32),
        "Wq": rng.standard_normal((E, E), np.float32) * 0.02,
        "bq": rng.standard_normal((E,), np.float32) * 0.02,
        "Wk": rng.standard_normal((E, E), np.float32) * 0.02,
        "bk": rng.standard_normal((E,), np.float32) * 0.02,
        "Wv": rng.standard_normal((E, E), np.float32) * 0.02,
        "bv": rng.standard_normal((E,), np.float32) * 0.02,
    }
    o1, o2 = kernel(**ins)
    print("ok", o1.shape, o2.shape, o1.dtype)
